# revision 19
# baseline (speedup 1.0000x reference)
"""Trainium2 Bass kernel for nn_Attention_36644660969693.

Multi-head attention block: x[8,32,32,768] -> qkv -> 12-head attention -> wo.
Sharding: data-parallel over batch, one image (1024 tokens) per NeuronCore;
no collectives.

Per-core design (T=1024 tokens, C=768, 12 heads, hd=64), all matmuls fp16
with fp32 PSUM accumulation:
  - x arrives via HWDGE (sync-engine) f32 DMA in four 256-token chunks so
    the SWDGE Q7 is free to generate weight-DMA descriptors concurrently;
    weight DMAs are ct-fused (one descriptor-gen op per column group) and
    ordered k0-cols, q0-cols, v-cols, bulk, wo by first-use time
  - a PE warmup loop (garbage matmuls) + a dummy exp run during the DMA
    wait so the HAM clock-gate is at 2.4 GHz and the exp table is loaded
    before real work starts
  - xT[c,t] via PE transpose of the f32 chunks; the PSUM->SBUF copy casts
    to fp16
  - qkT[f,t] = w_qkv tile-stationary @ xT; head h lands at partition
    (h*64)%128 of f-tile h//2, so a head PAIR occupies the two partition
    halves of one tile.  f6-ch0 (keys for the first 512 tokens) is
    emitted as soon as x chunks 0-1 are transposed so pair-0 scores and
    exp start ~12 us in; chunks 2-3 + f6-ch1 + f0-ch1 are injected
    mid-way through pair 0's first half-pass
  - v[t,f] natural orientation, stored per pair as [v_even | ones | v_odd]
    (192 cols): the AV stationary for the even head is [v|1], for the odd
    head [1|v], so each AV matmul emits the softmax row-sums in the
    complementary 64 output partitions for free; v for pairs 0-3 is
    computed inside pair 0's first half-pass (shifted one key-tile late so
    the v-weight DMA has landed), pairs 4-5's v in pairs 1-2's filler
  - scoresT[j,i] per head = kT-tile-stationary @ qT (K=64): the two packed
    heads write the two banks of ONE psum tile back-to-back (disjoint PE
    row groups -> they execute concurrently), and a single ScalarE exp per
    (key-tile, i-chunk) reads the pair straight from PSUM with the 1/8
    scale fused, writing fp16
  - each pair runs as two i-chunk half-passes with 2-key-tile batched
    score groups; AV accumulation lags exp by two key tiles, and each
    half-pass's final AV steps + normalize are deferred into the next
    half-pass so ScalarE never stalls at boundaries; next-pair qkT tiles
    and (for the last pair) the first half of the output projection
    interleave as fine-grained PE filler (2-6 matmuls per slot)
  - normalize = full-partition fast-reciprocal of the row-sum half +
    64-partition swap DMA + elementwise multiply into aT[c,t] (fp16)
  - out[t,:] = aT-tile-stationary @ w_o, natural layout, DMA straight out
The pipeline is TensorE-work-bound end to end; every remaining component
sits at its forced contraction-steps x N-cycles floor for this layout.
"""

import numpy as np

import concourse.bass as bass
import concourse.tile as tile
from concourse import bacc, mybir
from concourse import bass_utils
from concourse import masks

P = 128          # partitions
T = 1024         # tokens per image
C = 768          # model dim
NT = T // P      # 8 token tiles
NC = C // P      # 6 channel tiles
NH = 12          # heads
HD = 64          # head dim
NPAIR = NH // 2  # 6 head pairs
VPW = 192        # v_pad pair block width: [v_even(64) | ones(64) | v_odd(64)]
SCALE = HD ** -0.5
F32 = mybir.dt.float32
F16 = mybir.dt.float16
EXP = mybir.ActivationFunctionType.Exp
NCHUNK = 4       # x DMA chunks (256 tokens each)
CTOK = T // NCHUNK


def attention_kernel(tc, out_d, x_d, wq_d, wo_d):
    nc = tc.nc
    from contextlib import ExitStack

    with ExitStack() as ctx:
        const_pool = ctx.enter_context(tc.tile_pool(name="const", bufs=1))
        persist = ctx.enter_context(tc.tile_pool(name="persist", bufs=1))
        opool = ctx.enter_context(tc.tile_pool(name="ot", bufs=2))

        identf = const_pool.tile([P, P], F32, tag="identf")
        warm = const_pool.tile([P, P], F16, tag="warm")
        actw = const_pool.tile([P, 8], F32, tag="actw")
        fencet = const_pool.tile([P, 8], F32, tag="fencet")

        xT = persist.tile([P, NC * T], F16, tag="xT")        # [c, t] blocks
        wq = persist.tile([P, NC * 2304], F16, tag="wq")     # [c, f] blocks
        qkT = persist.tile([P, 12 * T], F16, tag="qkT")      # [f, t] blocks
        vpad = persist.tile([P, NT * NPAIR * VPW], F16, tag="vpad")
        aT = persist.tile([P, NC * T], F16, tag="aT")        # [c, t] blocks
        wo_sb = persist.tile([P, NC * C], F16, tag="wo")     # [c, c'] blocks

        # ---- input DMAs ----
        # x via HWDGE (sync engine, f32): 4 chunks of 256 tokens, issued
        # up front; SWDGE Q7 generates weight descriptors in parallel.
        xst = ctx.enter_context(tc.tile_pool(name="xst", bufs=3))
        xh32 = []
        for cb in range(NCHUNK):
            xh = xst.tile([P, 2 * C], F32, tag="xh32", name=f"xh{cb}")
            src = x_d[cb * CTOK:(cb + 1) * CTOK, :].rearrange(
                "(k p) c -> p k c", p=P
            )
            # alternate the two HWDGE rings so x holds 2 of the 3 active
            # DMA rings (SWDGE weights being the third)
            eng = nc.sync if cb % 2 == 0 else nc.scalar
            eng.dma_start(xh[:].rearrange("p (k c) -> p k c", k=2), src)
            xh32.append(xh)

        # identity before weight DMAs so transposes unblock early
        masks.make_identity(nc, identf[:])

        # weight DMAs: SWDGE casting f32->f16, ct-fused (one call per
        # column group), ordered by first use.
        def wq_fused_dma(off, pattern):
            w0 = wq[:]
            dst = bass.AP(tensor=w0.tensor, offset=w0.offset + off,
                          ap=[w0.ap[0], [2304, NC]] + pattern)
            src = bass.AP(tensor=wq_d.tensor, offset=wq_d.offset + off,
                          ap=[[2304, P], [P * 2304, NC]] + pattern)
            nc.gpsimd.dma_start(dst, src)

        def dma_fence(xh):
            """Q7-side throttle: descriptor generation for every weight DMA
            issued after this blocks until the given x chunk has landed, so
            the x stream keeps near-exclusive HBM bandwidth."""
            nc.gpsimd.dma_start(fencet[0:1, 0:8], xh[0:1, 0:8])

        wq_fused_dma(768, [[1, 128]])            # k cols of pair 0 (f6)
        dma_fence(xh32[0])
        wq_fused_dma(0, [[1, 128]])              # q cols of pair 0 (f0)
        wq_fused_dma(1536, [[1, 128]])           # v cols of pair 0
        dma_fence(xh32[2])
        wq_fused_dma(1664, [[1, 384]])           # v cols, pairs 1-3
        wq_fused_dma(128, [[1, 128]])            # q cols of pair 1 (f1)
        wq_fused_dma(896, [[1, 128]])            # k cols of pair 1 (f7)
        wq_fused_dma(2048, [[1, 256]])           # v cols, pairs 4-5
        wq_fused_dma(256, [[1, 512]])            # q cols, pairs 2-5
        wq_fused_dma(1024, [[1, 512]])           # k cols, pairs 2-5
        w0 = wo_sb[:]
        dst = bass.AP(tensor=w0.tensor, offset=w0.offset,
                      ap=[w0.ap[0], [C, NC], [1, C]])
        src = bass.AP(tensor=wo_d.tensor, offset=wo_d.offset,
                      ap=[[C, P], [P * C, NC], [1, C]])
        nc.gpsimd.dma_start(dst, src)

        # ---- engine warmups (overlap the x DMA wait) ----
        nc.vector.memset(warm[:], 0.25)
        nc.scalar.activation(actw[:], warm[:, 0:8], EXP, scale=SCALE)

        prep_ctx = ExitStack()
        pp_prep = prep_ctx.enter_context(
            tc.tile_pool(name="pprep", bufs=2, space="PSUM"))

        wps = pp_prep.tile([P, P], F32, tag="wu", name="wups")
        for _ in range(44):
            nc.tensor.matmul(wps[:], warm[:], warm[:], start=True, stop=True)

        # ---- x transposes: per chunk, per ct-pair; the PSUM->SBUF cast
        # copies are split between ScalarE and VectorE so neither engine's
        # queue serializes the prefix ----
        def emit_chunk_T(cb, tr_pool, tr_tag):
            xh = xh32[cb]
            for cp in range(NC // 2):
                ps = tr_pool.tile([P, 512], F32, tag=tr_tag, name="ps_tr")
                for m in range(2):
                    ct = 2 * cp + m
                    for k in range(2):
                        nc.tensor.transpose(
                            ps[:, m * 256 + k * P: m * 256 + (k + 1) * P],
                            xh[:, k * C + ct * P: k * C + ct * P + P],
                            identf[:],
                        )
                for m in range(2):
                    ct = 2 * cp + m
                    dst_off = ct * T + cb * CTOK
                    dst = xT[:, dst_off: dst_off + CTOK]
                    src = ps[:, m * 256: m * 256 + CTOK]
                    if cp < 2:
                        nc.scalar.copy(dst, src)
                    else:
                        nc.vector.tensor_copy(dst, src)

        emit_chunk_T(0, pp_prep, "tr")
        emit_chunk_T(1, pp_prep, "tr")

        qk_psum = {"pool": pp_prep, "tag": "mm"}

        def make_qk_emitter(ft, chs=(0, 1)):
            """Returns step(n): emits n accumulation matmuls of the qkT
            f-tile computation for the given i-chunks, so the work
            interleaves finely with the score stream."""
            pool, tag = qk_psum["pool"], qk_psum["tag"]
            st = {"ci": 0, "ct": 0, "ps": None}

            def step(n):
                for _ in range(n):
                    ci, ct = st["ci"], st["ct"]
                    if ci >= len(chs):
                        return
                    ch = chs[ci]
                    if ct == 0:
                        st["ps"] = pool.tile([P, 512], F32, tag=tag, name="ps_qk")
                    nc.tensor.matmul(
                        st["ps"][:],
                        wq[:, ct * 2304 + ft * P: ct * 2304 + ft * P + P],
                        xT[:, ct * T + ch * 512: ct * T + ch * 512 + 512],
                        start=(ct == 0),
                        stop=(ct == NC - 1),
                    )
                    if ct == NC - 1:
                        nc.vector.tensor_copy(
                            qkT[:, ft * T + ch * 512: ft * T + ch * 512 + 512],
                            st["ps"][:])
                        st["ci"], st["ct"] = ci + 1, 0
                    else:
                        st["ct"] = ct + 1
            return step

        # keys (f6) and queries (f0) for the first 512 tokens: everything
        # pair-0 ch0 scores need.
        make_qk_emitter(6, (0,))(NC)
        make_qk_emitter(0, (0,))(NC)
        prep_ctx.close()

        # ---- attention, one head pair at a time ----
        epool = ctx.enter_context(tc.tile_pool(name="E", bufs=2))
        rpool = ctx.enter_context(tc.tile_pool(name="recip", bufs=2))
        pp_s = ctx.enter_context(tc.tile_pool(name="pps", bufs=2, space="PSUM"))
        pp_av = ctx.enter_context(tc.tile_pool(name="ppav", bufs=4, space="PSUM"))
        qk_psum["pool"], qk_psum["tag"] = pp_av, "av"

        def inject_late_prefix():
            """Mid-pair-0 injection: transpose x chunks 2-3 and finish the
            pair-0 qk tiles (keys f6-ch1)."""
            emit_chunk_T(2, pp_av, "av")
            emit_chunk_T(3, pp_av, "av")
            make_qk_emitter(6, (1,))(NC)

        wo_state = {}

        def emit_wo_half(tt, half):
            """Half of one output t-tile projection (6 matmuls)."""
            if half == 0:
                po1 = pp_av.tile([P, 512], F32, tag="av", name="po1")
                wo_state[tt] = po1
                for ct in range(NC):
                    lhsT = aT[:, ct * T + tt * P: ct * T + tt * P + P]
                    nc.tensor.matmul(po1[:], lhsT, wo_sb[:, ct * C: ct * C + 512],
                                     start=(ct == 0), stop=(ct == NC - 1))
            else:
                po1 = wo_state.pop(tt)
                po2 = pp_av.tile([P, 512], F32, tag="av", name="po2")
                for ct in range(NC):
                    lhsT = aT[:, ct * T + tt * P: ct * T + tt * P + P]
                    nc.tensor.matmul(po2[:, :256], lhsT,
                                     wo_sb[:, ct * C + 512: ct * C + C],
                                     start=(ct == 0), stop=(ct == NC - 1))
                ot = opool.tile([P, C], F32, tag="ot", name="ot")
                nc.scalar.copy(ot[:, 0:512], po1[:])
                nc.vector.tensor_copy(ot[:, 512:C], po2[:, :256])
                nc.sync.dma_start(out_d[tt * P:(tt + 1) * P, :], ot[:])

        wo_part = {}

        def emit_wo_partial(tt, use_s_pool):
            """ct 0-4 of both output halves of tile tt; fills the PE while
            the final pair's normalize chain runs on DVE."""
            if use_s_pool:
                po = pp_s.tile([P, T], F32, tag="s", name=f"wop{tt}")
                p1, p2 = po[:, 0:512], po[:, 512:768]
            else:
                p1t = pp_av.tile([P, 512], F32, tag="av", name=f"wp1{tt}")
                p2t = pp_av.tile([P, 512], F32, tag="av", name=f"wp2{tt}")
                p1, p2 = p1t[:], p2t[:, :256]
            wo_part[tt] = (p1, p2)
            for ct in range(NC - 1):
                lhsT = aT[:, ct * T + tt * P: ct * T + tt * P + P]
                nc.tensor.matmul(p1, lhsT, wo_sb[:, ct * C: ct * C + 512],
                                 start=(ct == 0), stop=False)
            for ct in range(NC - 1):
                lhsT = aT[:, ct * T + tt * P: ct * T + tt * P + P]
                nc.tensor.matmul(p2, lhsT, wo_sb[:, ct * C + 512: ct * C + C],
                                 start=(ct == 0), stop=False)

        def emit_wo_final(tt):
            p1, p2 = wo_part.pop(tt)
            ct = NC - 1
            lhsT = aT[:, ct * T + tt * P: ct * T + tt * P + P]
            nc.tensor.matmul(p1, lhsT, wo_sb[:, ct * C: ct * C + 512],
                             start=False, stop=True)
            nc.tensor.matmul(p2, lhsT, wo_sb[:, ct * C + 512: ct * C + C],
                             start=False, stop=True)
            ot = opool.tile([P, C], F32, tag="ot", name="ot")
            nc.scalar.copy(ot[:, 0:512], p1)
            nc.vector.tensor_copy(ot[:, 512:C], p2)
            eng = nc.sync if tt % 2 == 0 else nc.scalar
            eng.dma_start(out_d[tt * P:(tt + 1) * P, :], ot[:])

        def vslice(jt, hp, h):
            """[128,128] AV stationary: even head [v|1], odd head [1|v]."""
            base = jt * NPAIR * VPW + hp * VPW + (0 if h == 0 else HD)
            return vpad[:, base: base + P]

        def emit_v_chunk(tt, foff, fw):
            """One 6-matmul chunk of v[t-tile tt] covering head pairs
            foff//128 .. (foff+fw)//128 - 1."""
            ps = pp_av.tile([P, 512], F32, tag="av", name="ps_v")
            for ct in range(NC):
                nc.tensor.matmul(
                    ps[:, :fw],
                    xT[:, ct * T + tt * P: ct * T + tt * P + P],
                    wq[:, ct * 2304 + 1536 + foff: ct * 2304 + 1536 + foff + fw],
                    start=(ct == 0),
                    stop=(ct == NC - 1),
                )
            npr = fw // 128
            src = ps[:, :fw].rearrange("p (m two d) -> p m two d", two=2, d=HD)
            base = tt * NPAIR * VPW + (foff // 128) * VPW
            dst = vpad[:, base: base + npr * VPW].rearrange(
                "p (m blk) -> p m blk", blk=VPW
            )
            nc.vector.tensor_copy(dst[:, :, 0:HD], src[:, :, 0, :])
            nc.vector.tensor_copy(dst[:, :, 2 * HD:VPW], src[:, :, 1, :])
            nc.vector.memset(dst[:, :, HD:2 * HD], 1.0)

        def normalize(a, h, hp, ch):
            r = rpool.tile([P, 512], F32, tag="r", name=f"r{h}{ch}")
            r2 = rpool.tile([P, 512], F32, tag="r2", name=f"r2{h}{ch}")
            dst = aT[:, hp * T + ch * 512: hp * T + ch * 512 + 512]
            # full-partition approx reciprocal (custom DVE op needs base
            # partition 0); the non-rowsum half of r is garbage, never read
            nc.vector.reciprocal_approx_fast(r[:, :], a[:, :])
            if h == 0:
                nc.sync.dma_start(r2[0:HD, :], r[HD:P, :])
                nc.vector.tensor_mul(dst[0:HD, :], a[0:HD, :], r2[0:HD, :])
            else:
                nc.sync.dma_start(r2[HD:P, :], r[0:HD, :])
                nc.vector.tensor_mul(dst[HD:P, :], a[HD:P, :], r2[HD:P, :])

        def eoff(jt, ch, h):
            return jt * 2048 + ch * T + h * 512

        def emit_scores_exp(hp, jt, ch, E):
            """Both packed heads' scores for one i-chunk into ONE 2-bank
            psum tile (forces the row-group pair to issue back-to-back),
            then a single exp over the pair."""
            qblk = hp * T
            kblk = (6 + hp) * T
            s = pp_s.tile([P, T], F32, tag="s", name="s")
            nc.tensor.matmul(
                s[:, 0:512],
                qkT[0:HD, kblk + jt * P: kblk + jt * P + P],
                qkT[0:HD, qblk + ch * 512: qblk + ch * 512 + 512],
                start=True, stop=True,
            )
            nc.tensor.matmul(
                s[:, 512:1024],
                qkT[HD:P, kblk + jt * P: kblk + jt * P + P],
                qkT[HD:P, qblk + ch * 512: qblk + ch * 512 + 512],
                start=True, stop=True,
            )
            nc.scalar.activation(E[:, eoff(jt, ch, 0): eoff(jt, ch, 0) + T],
                                 s[:], EXP, scale=SCALE)

        pending_tail = None   # previous half-pass: final AV steps + normalizes

        for hp in range(NPAIR):
            E = epool.tile([P, NT * 2048], F16, tag="E", name="E")
            last = hp == NPAIR - 1

            for ch in range(2):
                a0 = pp_av.tile([P, 512], F32, tag="av", name=f"a0c{ch}")
                a1 = pp_av.tile([P, 512], F32, tag="av", name=f"a1c{ch}")

                def av_step(jt, ch=ch, a0=a0, a1=a1, hp=hp, E=E):
                    for a, h in ((a0, 0), (a1, 1)):
                        nc.tensor.matmul(
                            a[:],
                            vslice(jt, hp, h),
                            E[:, eoff(jt, ch, h): eoff(jt, ch, h) + 512],
                            start=(jt == 0),
                            stop=(jt == NT - 1),
                        )

                # fine-grained filler schedule for this half-pass
                vq = []
                if hp == 0 and ch == 1:
                    qk_steps = [make_qk_emitter(1), make_qk_emitter(7)]
                    # v for pairs 1-3 (weights arrive mid pair 0)
                    vq = [(tt, 1 * P, 384) for tt in range(NT)]
                elif 0 < hp < NPAIR - 1:
                    qk_steps = [make_qk_emitter(hp + 1 if ch == 0 else 6 + hp + 1)]
                    if hp <= 2:
                        # v for pairs 4-5 (heads 8-11), 2 t-tiles per
                        # half-pass across pairs 1-2
                        base = (hp - 1) * 4 + ch * 2
                        vq = [(base, 4 * P, 256), (base + 1, 4 * P, 256)]
                else:
                    qk_steps = []

                for jtp in range(0, NT, 2):
                    if hp == 0 and ch == 0 and jtp == 4:
                        inject_late_prefix()
                    # two adjacent score-pair groups: their stationaries sit
                    # in disjoint PE row groups, so weight loads pre-overlap
                    for jt in (jtp, jtp + 1):
                        emit_scores_exp(hp, jt, ch, E)
                        if jt == 0 and pending_tail is not None:
                            pending_tail()
                            pending_tail = None
                        if hp == 0 and ch == 0 and jt >= 1:
                            # pair-0 v only: its 128 weight cols land early
                            emit_v_chunk(jt - 1, 0, 128)
                    for jt in (jtp, jtp + 1):
                        if jt >= 2:
                            av_step(jt - 2)
                        for q in qk_steps:
                            q(2)
                        if vq and (hp > 0 and jt in (3, 6) or hp == 0):
                            emit_v_chunk(*vq.pop(0))
                        if last and ch == 1 and jt >= 1:
                            emit_wo_half((jt - 1) // 2, (jt - 1) % 2)
                if hp == 0 and ch == 0:
                    emit_v_chunk(NT - 1, 0, 128)
                    # queries f0-ch1 for pair 0's second half-pass
                    make_qk_emitter(0, (1,))(NC)
                for q in qk_steps:
                    q(2 * NC)   # drain any remainder

                def make_tail(av_step=av_step, a0=a0, a1=a1, hp=hp, ch=ch):
                    def run():
                        av_step(NT - 2)
                        av_step(NT - 1)
                        normalize(a0, 0, hp, ch)
                        normalize(a1, 1, hp, ch)
                    return run

                pending_tail = make_tail()

        pending_tail()
        emit_wo_half(3, 1)
        for tt in range(NT // 2, NT):
            emit_wo_partial(tt, use_s_pool=(tt < 6))
        for tt in range(NT // 2, NT):
            emit_wo_final(tt)


_CACHED = {}
def build_program():
    if "nc" in _CACHED:
        return _CACHED["nc"]
    nc = bacc.Bacc("TRN2", target_bir_lowering=False, debug=False, num_devices=8)
    x_d = nc.dram_tensor("x", [T, C], F32, kind="ExternalInput").ap()
    wq_d = nc.dram_tensor("w_qkv", [C, 3 * C], F32, kind="ExternalInput").ap()
    wo_d = nc.dram_tensor("w_o", [C, C], F32, kind="ExternalInput").ap()
    out_d = nc.dram_tensor("out", [T, C], F32, kind="ExternalOutput").ap()
    with tile.TileContext(nc) as tc:
        attention_kernel(tc, out_d, x_d, wq_d, wo_d)
    nc.compile()
    _CACHED["nc"] = nc
    return nc


def kernel(x, w_qkv, w_o, _trace=False, _trace_cores=None):
    nc = build_program()
    x = np.ascontiguousarray(np.asarray(x, dtype=np.float32))
    w_qkv = np.ascontiguousarray(np.asarray(w_qkv, dtype=np.float32))
    w_o = np.ascontiguousarray(np.asarray(w_o, dtype=np.float32))
    bs = x.shape[0]
    in_maps = [
        {"x": x[b].reshape(T, C), "w_qkv": w_qkv, "w_o": w_o} for b in range(bs)
    ]
    res = bass_utils.run_bass_kernel_spmd(
        nc, in_maps, core_ids=list(range(bs)), trace=_trace,
        trace_cores=_trace_cores,
    )
    out = np.stack([res.results[b]["out"].reshape(32, 32, C) for b in range(bs)])
    if _trace:
        return out, res
    return out


# revision 21
# speedup vs baseline: 1.1726x; 1.1726x over previous
"""Trainium2 Bass kernel for nn_Attention_36644660969693.

Multi-head attention block: x[8,32,32,768] -> qkv -> 12-head attention -> wo.
Sharding: data-parallel over batch, one image (1024 tokens) per NeuronCore;
no collectives.

Per-core design (T=1024 tokens, C=768, 12 heads, hd=64), all matmuls fp16
with fp32 PSUM accumulation:
  - x arrives via HWDGE (sync-engine) f32 DMA in four 256-token chunks so
    the SWDGE Q7 is free to generate weight-DMA descriptors concurrently;
    weight DMAs are ct-fused (one descriptor-gen op per column group) and
    ordered k0-cols, q0-cols, v-cols, bulk, wo by first-use time
  - a PE warmup loop (garbage matmuls) + a dummy exp run during the DMA
    wait so the HAM clock-gate is at 2.4 GHz and the exp table is loaded
    before real work starts
  - xT[c,t] via PE transpose of the f32 chunks; the PSUM->SBUF copy casts
    to fp16
  - qkT[f,t] = w_qkv tile-stationary @ xT; head h lands at partition
    (h*64)%128 of f-tile h//2, so a head PAIR occupies the two partition
    halves of one tile.  f6-ch0 (keys for the first 512 tokens) is
    emitted as soon as x chunks 0-1 are transposed so pair-0 scores and
    exp start ~12 us in; chunks 2-3 + f6-ch1 + f0-ch1 are injected
    mid-way through pair 0's first half-pass
  - v[t,f] natural orientation, stored per pair as [v_even | ones | v_odd]
    (192 cols): the AV stationary for the even head is [v|1], for the odd
    head [1|v], so each AV matmul emits the softmax row-sums in the
    complementary 64 output partitions for free; v for pairs 0-3 is
    computed inside pair 0's first half-pass (shifted one key-tile late so
    the v-weight DMA has landed), pairs 4-5's v in pairs 1-2's filler
  - scoresT[j,i] per head = kT-tile-stationary @ qT (K=64): the two packed
    heads write the two banks of ONE psum tile back-to-back (disjoint PE
    row groups -> they execute concurrently), and a single ScalarE exp per
    (key-tile, i-chunk) reads the pair straight from PSUM with the 1/8
    scale fused, writing fp16
  - each pair runs as two i-chunk half-passes with 2-key-tile batched
    score groups; AV accumulation lags exp by two key tiles, and each
    half-pass's final AV steps + normalize are deferred into the next
    half-pass so ScalarE never stalls at boundaries; next-pair qkT tiles
    and (for the last pair) the first half of the output projection
    interleave as fine-grained PE filler (2-6 matmuls per slot)
  - normalize = full-partition fast-reciprocal of the row-sum half +
    64-partition swap DMA + elementwise multiply into aT[c,t] (fp16)
  - out[t,:] = aT-tile-stationary @ w_o, natural layout, DMA straight out
The pipeline is TensorE-work-bound end to end; every remaining component
sits at its forced contraction-steps x N-cycles floor for this layout.
"""

import numpy as np

import concourse.bass as bass
import concourse.tile as tile
from concourse import bacc, mybir
from concourse import bass_utils
from concourse import masks

P = 128          # partitions
T = 1024         # tokens per image
C = 768          # model dim
NT = T // P      # 8 token tiles
NC = C // P      # 6 channel tiles
NH = 12          # heads
HD = 64          # head dim
NPAIR = NH // 2  # 6 head pairs
VPW = 192        # v_pad pair block width: [v_even(64) | ones(64) | v_odd(64)]
SCALE = HD ** -0.5
F32 = mybir.dt.float32
F16 = mybir.dt.float16
F32R = mybir.dt.float32r
EXP = mybir.ActivationFunctionType.Exp
NCHUNK = 4       # x DMA chunks (256 tokens each)
CTOK = T // NCHUNK


def attention_kernel(tc, out_d, x_d, wq_d, wo_d):
    nc = tc.nc
    from contextlib import ExitStack

    with ExitStack() as ctx:
        const_pool = ctx.enter_context(tc.tile_pool(name="const", bufs=1))
        persist = ctx.enter_context(tc.tile_pool(name="persist", bufs=1))
        opool = ctx.enter_context(tc.tile_pool(name="ot", bufs=4))

        identf = const_pool.tile([P, P], F32, tag="identf")
        warm = const_pool.tile([P, P], F16, tag="warm")
        actw = const_pool.tile([P, 8], F32, tag="actw")
        fencet = const_pool.tile([P, 8], F32, tag="fencet")

        xT = persist.tile([P, NC * T], F16, tag="xT")        # [c, t] blocks
        wq = persist.tile([P, NC * 2304], F16, tag="wq")     # [c, f] blocks
        qkT = persist.tile([P, 12 * T], F16, tag="qkT")      # [f, t] blocks
        vpad = persist.tile([P, NT * NPAIR * VPW], F16, tag="vpad")
        aT = persist.tile([P, NC * T], F16, tag="aT")        # [c, t] blocks
        wo_sb = persist.tile([P, NC * C], F16, tag="wo")     # [c, c'] blocks

        # ---- input DMAs ----
        # x via HWDGE (sync engine, f32): 4 chunks of 256 tokens, issued
        # up front; SWDGE Q7 generates weight descriptors in parallel.
        xst = ctx.enter_context(tc.tile_pool(name="xst", bufs=4))
        xh32 = []
        for cb in range(NCHUNK):
            xh = xst.tile([P, 2 * C], F32, tag="xh32", name=f"xh{cb}")
            src = x_d[cb * CTOK:(cb + 1) * CTOK, :].rearrange(
                "(k p) c -> p k c", p=P
            )
            # alternate the two HWDGE rings so x holds 2 of the 3 active
            # DMA rings (SWDGE weights being the third)
            eng = nc.sync if cb % 2 == 0 else nc.scalar
            eng.dma_start(xh[:].rearrange("p (k c) -> p k c", k=2), src)
            xh32.append(xh)

        # identity before weight DMAs so transposes unblock early
        masks.make_identity(nc, identf[:])

        # weight DMAs: SWDGE casting f32->f16, ct-fused (one call per
        # column group), ordered by first use.
        def wq_fused_dma(off, pattern):
            w0 = wq[:]
            dst = bass.AP(tensor=w0.tensor, offset=w0.offset + off,
                          ap=[w0.ap[0], [2304, NC]] + pattern)
            src = bass.AP(tensor=wq_d.tensor, offset=wq_d.offset + off,
                          ap=[[2304, P], [P * 2304, NC]] + pattern)
            nc.gpsimd.dma_start(dst, src)

        def dma_fence(xh):
            """Q7-side throttle: descriptor generation for every weight DMA
            issued after this blocks until the given x chunk has landed, so
            the x stream keeps near-exclusive HBM bandwidth."""
            nc.gpsimd.dma_start(fencet[0:1, 0:8], xh[0:1, 0:8])

        wq_fused_dma(768, [[1, 128]])            # k cols of pair 0 (f6)
        dma_fence(xh32[0])
        wq_fused_dma(0, [[1, 128]])              # q cols of pair 0 (f0)
        wq_fused_dma(1536, [[1, 128]])           # v cols of pair 0
        dma_fence(xh32[2])
        wq_fused_dma(1664, [[1, 384]])           # v cols, pairs 1-3
        wq_fused_dma(128, [[1, 128]])            # q cols of pair 1 (f1)
        wq_fused_dma(896, [[1, 128]])            # k cols of pair 1 (f7)
        wq_fused_dma(2048, [[1, 256]])           # v cols, pairs 4-5
        wq_fused_dma(256, [[1, 512]])            # q cols, pairs 2-5
        wq_fused_dma(1024, [[1, 512]])           # k cols, pairs 2-5
        w0 = wo_sb[:]
        dst = bass.AP(tensor=w0.tensor, offset=w0.offset,
                      ap=[w0.ap[0], [C, NC], [1, C]])
        src = bass.AP(tensor=wo_d.tensor, offset=wo_d.offset,
                      ap=[[C, P], [P * C, NC], [1, C]])
        nc.gpsimd.dma_start(dst, src)

        # ---- engine warmups (overlap the x DMA wait) ----
        nc.vector.memset(warm[:], 0.25)
        nc.scalar.activation(actw[:], warm[:, 0:8], EXP, scale=SCALE)

        prep_ctx = ExitStack()
        pp_prep = prep_ctx.enter_context(
            tc.tile_pool(name="pprep", bufs=2, space="PSUM"))

        wps = pp_prep.tile([P, P], F32, tag="wu", name="wups")
        for _ in range(32):
            nc.tensor.matmul(wps[:], warm[:], warm[:], start=True, stop=True)

        # ---- x transposes: per chunk, per ct-pair; the PSUM->SBUF cast
        # copies are split between ScalarE and VectorE so neither engine's
        # queue serializes the prefix ----
        def emit_chunk_T(cb, tr_pool, tr_tag):
            xh = xh32[cb]
            for cp in range(NC // 2):
                ps = tr_pool.tile([P, 512], F32, tag=tr_tag, name="ps_tr")
                for m in range(2):
                    ct = 2 * cp + m
                    for k in range(2):
                        nc.tensor.transpose(
                            ps[:, m * 256 + k * P: m * 256 + (k + 1) * P],
                            xh[:, k * C + ct * P: k * C + ct * P + P],
                            identf[:],
                        )
                for m in range(2):
                    ct = 2 * cp + m
                    dst_off = ct * T + cb * CTOK
                    dst = xT[:, dst_off: dst_off + CTOK]
                    src = ps[:, m * 256: m * 256 + CTOK]
                    if cp < 2:
                        nc.scalar.copy(dst, src)
                    else:
                        nc.vector.tensor_copy(dst, src)

        emit_chunk_T(0, pp_prep, "tr")
        emit_chunk_T(1, pp_prep, "tr")

        qk_psum = {"pool": pp_prep, "tag": "mm"}

        def make_qk_emitter(ft, chs=(0, 1)):
            """Returns step(n): emits n accumulation matmuls of the qkT
            f-tile computation for the given i-chunks, so the work
            interleaves finely with the score stream."""
            pool, tag = qk_psum["pool"], qk_psum["tag"]
            st = {"ci": 0, "ct": 0, "ps": None}

            def step(n):
                for _ in range(n):
                    ci, ct = st["ci"], st["ct"]
                    if ci >= len(chs):
                        return
                    ch = chs[ci]
                    if ct == 0:
                        st["ps"] = pool.tile([P, 512], F32, tag=tag, name="ps_qk")
                    nc.tensor.matmul(
                        st["ps"][:],
                        wq[:, ct * 2304 + ft * P: ct * 2304 + ft * P + P],
                        xT[:, ct * T + ch * 512: ct * T + ch * 512 + 512],
                        start=(ct == 0),
                        stop=(ct == NC - 1),
                    )
                    if ct == NC - 1:
                        nc.vector.tensor_copy(
                            qkT[:, ft * T + ch * 512: ft * T + ch * 512 + 512],
                            st["ps"][:])
                        st["ci"], st["ct"] = ci + 1, 0
                    else:
                        st["ct"] = ct + 1
            return step

        # keys (f6) and queries (f0) for the first 512 tokens: everything
        # pair-0 ch0 scores need.
        make_qk_emitter(6, (0,))(NC)
        make_qk_emitter(0, (0,))(NC)
        prep_ctx.close()

        # ---- attention, one head pair at a time ----
        epool = ctx.enter_context(tc.tile_pool(name="E", bufs=2))
        rpool = ctx.enter_context(tc.tile_pool(name="recip", bufs=2))
        pp_s = ctx.enter_context(tc.tile_pool(name="pps", bufs=2, space="PSUM"))
        pp_av = ctx.enter_context(tc.tile_pool(name="ppav", bufs=4, space="PSUM"))
        qk_psum["pool"], qk_psum["tag"] = pp_av, "av"

        def inject_late_prefix():
            """Mid-pair-0 injection: transpose x chunks 2-3 and finish the
            pair-0 qk tiles (keys f6-ch1)."""
            emit_chunk_T(2, pp_av, "av")
            emit_chunk_T(3, pp_av, "av")
            make_qk_emitter(6, (1,))(NC)

        wo_state = {}

        def emit_wo_half(tt, half):
            """Half of one output t-tile projection (6 matmuls)."""
            if half == 0:
                po1 = pp_av.tile([P, 512], F32, tag="av", name="po1")
                wo_state[tt] = po1
                for ct in range(NC):
                    lhsT = aT[:, ct * T + tt * P: ct * T + tt * P + P]
                    nc.tensor.matmul(po1[:], lhsT, wo_sb[:, ct * C: ct * C + 512],
                                     start=(ct == 0), stop=(ct == NC - 1))
            else:
                po1 = wo_state.pop(tt)
                po2 = pp_av.tile([P, 512], F32, tag="av", name="po2")
                for ct in range(NC):
                    lhsT = aT[:, ct * T + tt * P: ct * T + tt * P + P]
                    nc.tensor.matmul(po2[:, :256], lhsT,
                                     wo_sb[:, ct * C + 512: ct * C + C],
                                     start=(ct == 0), stop=(ct == NC - 1))
                ot = opool.tile([P, C], F16, tag="ot", name="ot")
                nc.scalar.copy(ot[:, 0:512], po1[:])
                nc.vector.tensor_copy(ot[:, 512:C], po2[:, :256])
                nc.gpsimd.dma_start(out_d[tt * P:(tt + 1) * P, :], ot[:])

        wo_part = {}

        def emit_wo_partial(tt, use_s_pool):
            """ct 0-4 of both output halves of tile tt; fills the PE while
            the final pair's normalize chain runs on DVE."""
            if use_s_pool:
                po = pp_s.tile([P, T], F32, tag="s", name=f"wop{tt}")
                p1, p2 = po[:, 0:512], po[:, 512:768]
            else:
                p1t = pp_av.tile([P, 512], F32, tag="av", name=f"wp1{tt}")
                p2t = pp_av.tile([P, 512], F32, tag="av", name=f"wp2{tt}")
                p1, p2 = p1t[:], p2t[:, :256]
            wo_part[tt] = (p1, p2)
            for ct in range(NC - 1):
                lhsT = aT[:, ct * T + tt * P: ct * T + tt * P + P]
                nc.tensor.matmul(p1, lhsT, wo_sb[:, ct * C: ct * C + 512],
                                 start=(ct == 0), stop=False)
            for ct in range(NC - 1):
                lhsT = aT[:, ct * T + tt * P: ct * T + tt * P + P]
                nc.tensor.matmul(p2, lhsT, wo_sb[:, ct * C + 512: ct * C + C],
                                 start=(ct == 0), stop=False)

        def emit_wo_final(tt):
            p1, p2 = wo_part.pop(tt)
            ct = NC - 1
            lhsT = aT[:, ct * T + tt * P: ct * T + tt * P + P]
            nc.tensor.matmul(p1, lhsT, wo_sb[:, ct * C: ct * C + 512],
                             start=False, stop=True)
            nc.tensor.matmul(p2, lhsT, wo_sb[:, ct * C + 512: ct * C + C],
                             start=False, stop=True)
            ot = opool.tile([P, C], F16, tag="ot", name="ot")
            nc.scalar.copy(ot[:, 0:512], p1)
            nc.vector.tensor_copy(ot[:, 512:C], p2)
            nc.gpsimd.dma_start(out_d[tt * P:(tt + 1) * P, :], ot[:])

        def vslice(jt, hp, h):
            """[128,128] AV stationary: even head [v|1], odd head [1|v]."""
            base = jt * NPAIR * VPW + hp * VPW + (0 if h == 0 else HD)
            return vpad[:, base: base + P]

        def emit_v_chunk(tt, foff, fw):
            """One 6-matmul chunk of v[t-tile tt] covering head pairs
            foff//128 .. (foff+fw)//128 - 1."""
            ps = pp_av.tile([P, 512], F32, tag="av", name="ps_v")
            for ct in range(NC):
                nc.tensor.matmul(
                    ps[:, :fw],
                    xT[:, ct * T + tt * P: ct * T + tt * P + P],
                    wq[:, ct * 2304 + 1536 + foff: ct * 2304 + 1536 + foff + fw],
                    start=(ct == 0),
                    stop=(ct == NC - 1),
                )
            npr = fw // 128
            src = ps[:, :fw].rearrange("p (m two d) -> p m two d", two=2, d=HD)
            base = tt * NPAIR * VPW + (foff // 128) * VPW
            dst = vpad[:, base: base + npr * VPW].rearrange(
                "p (m blk) -> p m blk", blk=VPW
            )
            nc.vector.tensor_copy(dst[:, :, 0:HD], src[:, :, 0, :])
            nc.vector.tensor_copy(dst[:, :, 2 * HD:VPW], src[:, :, 1, :])
            nc.vector.memset(dst[:, :, HD:2 * HD], 1.0)

        def normalize(a, h, hp, ch):
            r = rpool.tile([P, 512], F32, tag="r", name=f"r{h}{ch}")
            r2 = rpool.tile([P, 512], F32, tag="r2", name=f"r2{h}{ch}")
            dst = aT[:, hp * T + ch * 512: hp * T + ch * 512 + 512]
            # full-partition approx reciprocal (custom DVE op needs base
            # partition 0); the non-rowsum half of r is garbage, never read
            nc.vector.reciprocal_approx_fast(r[:, :], a[:, :])
            if h == 0:
                nc.sync.dma_start(r2[0:HD, :], r[HD:P, :])
                nc.vector.tensor_mul(dst[0:HD, :], a[0:HD, :], r2[0:HD, :])
            else:
                nc.sync.dma_start(r2[HD:P, :], r[0:HD, :])
                nc.vector.tensor_mul(dst[HD:P, :], a[HD:P, :], r2[HD:P, :])

        def eoff(jt, ch, h):
            return jt * 2048 + ch * T + h * 512

        def emit_scores_exp(hp, jt, ch, E):
            """Both packed heads' scores for one i-chunk into ONE 2-bank
            psum tile (forces the row-group pair to issue back-to-back),
            then a single exp over the pair."""
            qblk = hp * T
            kblk = (6 + hp) * T
            s = pp_s.tile([P, T], F32, tag="s", name="s")
            nc.tensor.matmul(
                s[:, 0:512],
                qkT[0:HD, kblk + jt * P: kblk + jt * P + P],
                qkT[0:HD, qblk + ch * 512: qblk + ch * 512 + 512],
                start=True, stop=True,
            )
            nc.tensor.matmul(
                s[:, 512:1024],
                qkT[HD:P, kblk + jt * P: kblk + jt * P + P],
                qkT[HD:P, qblk + ch * 512: qblk + ch * 512 + 512],
                start=True, stop=True,
            )
            nc.scalar.activation(E[:, eoff(jt, ch, 0): eoff(jt, ch, 0) + T],
                                 s[:], EXP, scale=SCALE)

        pending_tail = None   # previous half-pass: final AV steps + normalizes

        for hp in range(NPAIR):
            E = epool.tile([P, NT * 2048], F16, tag="E", name="E")
            last = hp == NPAIR - 1

            for ch in range(2):
                a0 = pp_av.tile([P, 512], F32, tag="av", name=f"a0c{ch}")
                a1 = pp_av.tile([P, 512], F32, tag="av", name=f"a1c{ch}")

                def av_step(jt, ch=ch, a0=a0, a1=a1, hp=hp, E=E):
                    for a, h in ((a0, 0), (a1, 1)):
                        nc.tensor.matmul(
                            a[:],
                            vslice(jt, hp, h),
                            E[:, eoff(jt, ch, h): eoff(jt, ch, h) + 512],
                            start=(jt == 0),
                            stop=(jt == NT - 1),
                        )

                # fine-grained filler schedule for this half-pass
                vq = []
                if hp == 0 and ch == 1:
                    qk_steps = [make_qk_emitter(1), make_qk_emitter(7)]
                    # v for pairs 1-3 (weights arrive mid pair 0)
                    vq = [(tt, 1 * P, 384) for tt in range(NT)]
                elif 0 < hp < NPAIR - 1:
                    qk_steps = [make_qk_emitter(hp + 1 if ch == 0 else 6 + hp + 1)]
                    if hp <= 2:
                        # v for pairs 4-5 (heads 8-11), 2 t-tiles per
                        # half-pass across pairs 1-2
                        base = (hp - 1) * 4 + ch * 2
                        vq = [(base, 4 * P, 256), (base + 1, 4 * P, 256)]
                else:
                    qk_steps = []

                for jtp in range(0, NT, 2):
                    if hp == 0 and ch == 0 and jtp == 4:
                        inject_late_prefix()
                    # two adjacent score-pair groups: their stationaries sit
                    # in disjoint PE row groups, so weight loads pre-overlap
                    for jt in (jtp, jtp + 1):
                        emit_scores_exp(hp, jt, ch, E)
                        if jt == 0 and pending_tail is not None:
                            pending_tail()
                            pending_tail = None
                        if hp == 0 and ch == 0 and jt >= 1:
                            # pair-0 v only: its 128 weight cols land early
                            emit_v_chunk(jt - 1, 0, 128)
                    for jt in (jtp, jtp + 1):
                        if jt >= 2:
                            av_step(jt - 2)
                        for q in qk_steps:
                            q(2)
                        if vq and (hp > 0 and jt in (3, 6) or hp == 0):
                            emit_v_chunk(*vq.pop(0))
                        if last and ch == 1 and jt >= 1:
                            emit_wo_half((jt - 1) // 2, (jt - 1) % 2)
                if hp == 0 and ch == 0:
                    emit_v_chunk(NT - 1, 0, 128)
                    # queries f0-ch1 for pair 0's second half-pass
                    make_qk_emitter(0, (1,))(NC)
                for q in qk_steps:
                    q(2 * NC)   # drain any remainder

                def make_tail(av_step=av_step, a0=a0, a1=a1, hp=hp, ch=ch):
                    def run():
                        av_step(NT - 2)
                        av_step(NT - 1)
                        normalize(a0, 0, hp, ch)
                        normalize(a1, 1, hp, ch)
                    return run

                pending_tail = make_tail()

        pending_tail()
        emit_wo_half(3, 1)
        for tt in range(NT // 2, NT):
            emit_wo_partial(tt, use_s_pool=(tt < 6))
        for tt in range(NT // 2, NT):
            emit_wo_final(tt)


_CACHED = {}
def build_program():
    if "nc" in _CACHED:
        return _CACHED["nc"]
    nc = bacc.Bacc("TRN2", target_bir_lowering=False, debug=False, num_devices=8)
    x_d = nc.dram_tensor("x", [T, C], F32, kind="ExternalInput").ap()
    wq_d = nc.dram_tensor("w_qkv", [C, 3 * C], F32, kind="ExternalInput").ap()
    wo_d = nc.dram_tensor("w_o", [C, C], F32, kind="ExternalInput").ap()
    out_d = nc.dram_tensor("out", [T, C], F32, kind="ExternalOutput").ap()
    with tile.TileContext(nc) as tc:
        attention_kernel(tc, out_d, x_d, wq_d, wo_d)
    nc.compile()
    _CACHED["nc"] = nc
    return nc


def kernel(x, w_qkv, w_o, _trace=False, _trace_cores=None):
    nc = build_program()
    x = np.ascontiguousarray(np.asarray(x, dtype=np.float32))
    w_qkv = np.ascontiguousarray(np.asarray(w_qkv, dtype=np.float32))
    w_o = np.ascontiguousarray(np.asarray(w_o, dtype=np.float32))
    bs = x.shape[0]
    in_maps = [
        {"x": x[b].reshape(T, C), "w_qkv": w_qkv, "w_o": w_o} for b in range(bs)
    ]
    res = bass_utils.run_bass_kernel_spmd(
        nc, in_maps, core_ids=list(range(bs)), trace=_trace,
        trace_cores=_trace_cores,
    )
    out = np.stack([res.results[b]["out"].reshape(32, 32, C) for b in range(bs)])
    if _trace:
        return out, res
    return out
